# revision 26
# baseline (speedup 1.0000x reference)
"""Trainium2 Bass kernel for a 4-layer transformer decoder (DP2 x TP4).

Strategy (v2):
  - 8 cores = 2 data-parallel groups of 4 tensor-parallel cores.
    Cores 0-3 process batch 0, cores 4-7 batch 1 (replica groups
    [[0,1,2,3],[4,5,6,7]]), 4 heads/core, FFN 1024/core, vocab
    8000/core.  Halves per-core collective bytes vs TP8 and lets the
    two groups' AllReduces run concurrently on disjoint links.
  - Each batch's 512 tokens are processed as 2 chunks of 256 so each
    sublayer-boundary AllReduce ([D, 256] bf16, 0.5MB) overlaps the
    other chunk's compute.
  - BatchNorm (inference) + biases folded into weights host-side;
    device residual z satisfies x = sigma*z + gamma with host-tracked
    sigma/gamma.  Boundary: z = sigma*z + allreduce(partial), one DVE
    scalar_tensor_tensor per (dt, chunk).
  - Residual + its direct consumers (qkv/ffn1/cross-q projections) in
    fp32r; everything downstream of a projection in bf16 with fp32
    PSUM accumulation.
  - Softmax in transposed layout [k, q]; denominator via a ones column
    augmented into the AV stationary operand; causal mask only on
    diagonal 128x128 blocks.
  - Cross-attn K/V for ALL layers precomputed right after layer-0
    self-attention to fill the initial collective-barrier wait.
"""

import sys
import numpy as np

if "/opt/trn_rl_repo" not in sys.path:
    sys.path.insert(0, "/opt/trn_rl_repo")

import ml_dtypes
import concourse.bass as bass
import concourse.mybir as mybir
import concourse.tile as tile
from concourse import bacc
from concourse import bass_utils

# model dims (hardcoded per spec)
V, D, H, L, B, S, SE = 32000, 1024, 16, 4, 2, 512, 512
DH = D // H            # 64
NC = 8                 # cores
NG = 2                 # data-parallel groups (one per batch)
TPC = NC // NG         # 4 tensor-parallel cores per group
HL = H // TPC          # 4 heads per core
EL = HL * DH           # 256 local head dims
ET = EL // 128         # 2 el-tiles
FF = 4 * D             # 4096
FFL = FF // TPC        # 1024 ffn hidden per core
HT = FFL // 128        # 8
VL = V // TPC          # 8000 vocab cols per core
VPAD = 8192
VS = VPAD // 128       # 64 vocab slices
TB = S                 # 512 tokens per batch (per group)
CH = 2                 # chunks per batch
CW = TB // CH          # 256 chunk width
DT = D // 128          # 8 d-tiles
KB = TB // 128         # 4 key blocks
NBND = 3 * L           # 12 boundaries

F32R = mybir.dt.float32r
F32 = mybir.dt.float32
BF16 = mybir.dt.bfloat16
AF = mybir.ActivationFunctionType
OP = mybir.AluOpType

RG = [[0, 1, 2, 3], [4, 5, 6, 7]]

# bias-tile column layout
COL_QKV = 0                     # L*12: l*12 + pj*2 + et   (pj: qs,ks,vs,qc,kc,vc)
COL_B1 = COL_QKV + 12 * L       # L*8: l*8 + ht
COL_SIG = COL_B1 + 8 * L        # 12*8: bnd*8 + dt
COL_BOUT = COL_SIG + 8 * NBND   # 64 cols
COL_EPS = COL_BOUT + VS
NBCOL = COL_EPS + 1


def _build_program():
    nc = bacc.Bacc("TRN2", target_bir_lowering=False, debug=False,
                   num_devices=NC)
    dd = lambda name, shape, dtype=F32R, kind="ExternalInput": \
        nc.dram_tensor(name, shape, dtype, kind=kind).ap()

    xt = dd("xt", [D, TB])
    enct = dd("enct", [D, TB], BF16)
    attw_s = dd("attw_s", [L, 128, 3 * EL * DT])     # col = dt*768+pj*256+e
    attq_c = dd("attq_c", [L, 128, EL * DT])         # col = dt*256 + e
    attkv_c = dd("attkv_c", [L, 128, 2 * EL * DT], BF16)  # col = dt*512+pj*256+e
    wo_s = dd("wo_s", [L, 128, ET * D], BF16)        # col = et*1024 + d
    wo_c = dd("wo_c", [L, 128, ET * D], BF16)
    w1p = dd("w1p", [L, 128, FFL * DT])              # col = dt*1024 + f
    w2p = dd("w2p", [L, 128, D * HT], BF16)          # col = ht*1024 + dout
    woutp = dd("woutp", [128, VS * D], BF16)         # col = vs*1024 + dt*128 + j
    biasp = dd("biasp", [128, NBCOL], F32)
    maskd = dd("maskd", [128, 128], BF16)            # strictly-lower 0/1
    identd = dd("identd", [128, 128], BF16)
    onesd = dd("onesd", [128, 64], BF16)
    logt = dd("logt", [VPAD, TB], BF16, kind="ExternalOutput")

    from contextlib import ExitStack
    with tile.TileContext(nc) as tc, ExitStack() as _es:
        P = lambda **kw: _es.enter_context(tc.tile_pool(**kw))
        cst = P(name="cst", bufs=1)
        zp = P(name="zp", bufs=1)
        zbp = P(name="zbp", bufs=1)
        encp = P(name="encp", bufs=1)
        ckvp = P(name="ckvp", bufs=1)
        kvwp = P(name="kvwp", bufs=1)
        wap = P(name="wap", bufs=1)
        waqc = P(name="waqc", bufs=1)
        wop = P(name="wop", bufs=1)
        w1pool = P(name="w1pool", bufs=1)
        w2pool = P(name="w2pool", bufs=1)
        wvp = P(name="wvp", bufs=2)
        qkvp = P(name="qkvp", bufs=2)
        ktp = P(name="ktp", bufs=1)
        svap = P(name="svap", bufs=1)
        esp = P(name="esp", bufs=10)
        hdp = P(name="hdp", bufs=2)
        csp = P(name="csp", bufs=2)
        hfp = P(name="hfp", bufs=1)
        aop = P(name="aop", bufs=2)
        arp = P(name="arp", bufs=2)
        rip = P(name="rip", bufs=2)
        osp = P(name="osp", bufs=2)
        ps = P(name="ps", bufs=6, space="PSUM")
        pst = P(name="pst", bufs=2, space="PSUM")
        dram = P(name="dram", bufs=3, space="DRAM")

        bias_sb = cst.tile([128, NBCOL], F32)
        nc.sync.dma_start(bias_sb[:], biasp[:])
        mask_sb = cst.tile([128, 128], BF16)
        nc.sync.dma_start(mask_sb[:], maskd[:])
        ident = cst.tile([128, 128], BF16)
        nc.sync.dma_start(ident[:], identd[:])
        ones_sb = cst.tile([128, 64], BF16)
        nc.sync.dma_start(ones_sb[:], onesd[:])
        zeros_sb = cst.tile([128, 128], BF16)
        nc.vector.tensor_scalar_mul(zeros_sb[:, 0:64], ones_sb[:], 0.0)
        nc.vector.tensor_copy(zeros_sb[:, 64:128], zeros_sb[:, 0:64])

        # residual stream (this core's batch), feature-major
        z = []
        for dt in range(DT):
            zt = zp.tile([128, TB], F32R, name=f"z{dt}")
            nc.sync.dma_start(zt[:], xt[dt * 128:(dt + 1) * 128, :])
            z.append(zt)

        zb = [zbp.tile([128, TB], BF16, name=f"zb{dt}") for dt in range(DT)]
        encs = []

        def bcol(c):
            return bias_sb[:, c:c + 1]

        def va_fill(va, vt_et, j, et):
            """Fill one et's blocks of an augmented-V tile from vt chunk j."""
            pt = pst.tile([128, 128], BF16, name="ptr")
            nc.tensor.transpose(pt[:], vt_et[:, j * 128:(j + 1) * 128],
                                ident[:])
            o = et * 192
            nc.vector.tensor_copy(va[:, o:o + 64], pt[:, 0:64])
            nc.vector.tensor_copy(va[:, o + 128:o + 192], pt[:, 64:128])
            nc.vector.tensor_copy(va[:, o + 64:o + 128], ones_sb[:])

        def attn_core(qt, kt, vaug, ch, causal, tag):
            """qt/kt: ET tiles [128, CW]/[128, TB]; vaug: KB tiles [128,384].
            Returns hd: ET bf16 tiles [128, CW]."""
            kbs = list(range(2 * (ch + 1))) if causal else list(range(KB))
            hd = [hdp.tile([128, CW], BF16, name=f"hd{et}")
                  for et in range(ET)]
            es_all = {}

            def scores(h):
                et, hh = h // 2, h % 2
                for kb in kbs:
                    rel = kb * 128 - ch * CW if causal else -128
                    q0 = max(rel, 0)
                    pp = ps.tile([128, CW], F32, name="psc", tag="mm")
                    nc.tensor.matmul(
                        pp[:],
                        kt[et][hh * 64:hh * 64 + 64, kb * 128:(kb + 1) * 128],
                        qt[et][hh * 64:hh * 64 + 64, :],
                        start=True, stop=True)
                    est = esp.tile([128, CW], BF16, name="es")
                    nc.scalar.activation(est[:, q0:CW], pp[:, q0:CW], AF.Exp)
                    if causal and rel >= 0:
                        if q0 > 0:
                            nc.vector.tensor_copy(est[:, 0:q0],
                                                  zeros_sb[:, 0:q0])
                        nc.vector.tensor_tensor(est[:, q0:q0 + 128],
                                                est[:, q0:q0 + 128],
                                                mask_sb[:], op=OP.mult)
                    es_all[(h, kb)] = est

            def av(h):
                et, hh = h // 2, h % 2
                po = ps.tile([128, CW], F32, name="po", tag="mm")
                st = (h // 2) * 192 + (h % 2) * 64
                for i, kb in enumerate(kbs):
                    nc.tensor.matmul(po[:], vaug[kb][:, st:st + 128],
                                     es_all[(h, kb)][:],
                                     start=(i == 0), stop=(i == len(kbs) - 1))
                nrows = po[0:64, :] if hh == 0 else po[64:128, :]
                crows = po[64:128, :] if hh == 0 else po[0:64, :]
                cs = csp.tile([64, CW], F32, name="cs")
                nc.scalar.activation(cs[:], crows, AF.Identity,
                                     bias=bias_sb[0:64, COL_EPS:COL_EPS + 1])
                rc = csp.tile([64, CW], F32, name="rc")
                nc.vector.reciprocal_approx_fast(out=rc[:], in_=cs[:])
                nc.vector.tensor_tensor(hd[et][hh * 64:hh * 64 + 64, :],
                                        nrows, rc[:], op=OP.mult)

            scores(0)
            for h in range(HL):
                if h + 1 < HL:
                    scores(h + 1)
                av(h)
            return hd

        def partial_ar(srcs, wsel, nsrc, ch):
            """Sum of wsel.T @ src across the 4-core group, via
            AllToAll + local DVE reduce + AllGather (cheaper than the
            cc-stack AllReduce, whose CCE reduce path halves DMA bw)."""
            arin = dram.tile([D, CW], BF16, name="arin")
            arout = dram.tile([D, CW], BF16, name="arout")
            ocw = aop.tile([128, DT * CW], BF16, name="ocw")
            for half in range(2):
                for dout in range(half * 4, half * 4 + 4):
                    pw = ps.tile([128, CW], F32, name="pw", tag="mm")
                    for i in range(nsrc):
                        c0 = i * D + dout * 128
                        nc.tensor.matmul(pw[:], wsel[:, c0:c0 + 128],
                                         srcs[i][:],
                                         start=(i == 0), stop=(i == nsrc - 1))
                    osl = ocw[:, dout * CW:(dout + 1) * CW]
                    if dout % 2 == 0:
                        nc.scalar.activation(osl, pw[:], AF.Copy)
                    else:
                        nc.vector.tensor_copy(osl, pw[:])
                h0 = half * 4
                nc.sync.dma_start(
                    arin[h0 * 128:(h0 + 4) * 128, :].rearrange(
                        "(dt p) t -> p dt t", p=128),
                    ocw[:, h0 * CW:(h0 + 4) * CW].rearrange(
                        "p (dt t) -> p dt t", t=CW))
            nc.gpsimd.collective_compute("AllReduce", OP.add,
                                         replica_groups=RG,
                                         ins=[arin[:]], outs=[arout[:]])
            return arout

        def boundary(arout, bnd, ch, last=False):
            art = arp.tile([128, DT * CW], BF16, name="art")
            for half in range(2):
                h0 = half * 4
                nc.sync.dma_start(
                    art[:, h0 * CW:(h0 + 4) * CW].rearrange(
                        "p (dt t) -> p dt t", t=CW),
                    arout[h0 * 128:(h0 + 4) * 128, :].rearrange(
                        "(dt p) t -> p dt t", p=128))
            cs_ = slice(ch * CW, (ch + 1) * CW)
            for dt in range(DT):
                dst = zb[dt][:, cs_] if last else z[dt][:, cs_]
                nc.vector.scalar_tensor_tensor(
                    dst, z[dt][:, cs_], bcol(COL_SIG + bnd * 8 + dt),
                    art[:, dt * CW:(dt + 1) * CW], OP.mult, OP.add)

        def zproj(stat_fn, ch):
            """psum[128,CW] = sum_dt stat(dt).T @ z[dt][:, chunk]."""
            pp = ps.tile([128, CW], F32, name="pj", tag="mm")
            for dt in range(DT):
                nc.tensor.matmul(pp[:], stat_fn(dt),
                                 z[dt][:, ch * CW:(ch + 1) * CW],
                                 start=(dt == 0), stop=(dt == DT - 1))
            return pp

        def cross_kv(l):
            akvc = kvwp.tile([128, 2 * EL * DT], BF16, name="akvc")
            nc.sync.dma_start(akvc[:], attkv_c[l])
            ktc_l, vtc_l = [], []
            for pj in range(2):          # 0=k, 1=v
                for et in range(ET):
                    pp = ps.tile([128, TB], F32, name="pkv", tag="mm")
                    for dt in range(DT):
                        c0 = dt * 2 * EL + pj * EL + et * 128
                        nc.tensor.matmul(pp[:], akvc[:, c0:c0 + 128],
                                         encs[dt][:],
                                         start=(dt == 0), stop=(dt == DT - 1))
                    nm = f"ktc{l}_{et}" if pj == 0 else f"vtc{et}"
                    out = ckvp.tile([128, TB], BF16, name=nm)
                    nc.scalar.activation(
                        out[:], pp[:], AF.Identity,
                        bias=bcol(COL_QKV + l * 12 + (4 + pj) * 2 + et))
                    (ktc_l if pj == 0 else vtc_l).append(out)
            vaug_l = []
            for kb in range(KB):
                va = ckvp.tile([128, 384], BF16, name=f"cva{l}_{kb}")
                for et in range(ET):
                    va_fill(va, vtc_l[et], kb, et)
                vaug_l.append(va)
            return ktc_l, vaug_l

        def ffn(l, w1t, w2t, ch):
            hts = []
            for ht in range(HT):
                pp = ps.tile([128, CW], F32, name="pf", tag="mm")
                for dt in range(DT):
                    c0 = dt * FFL + ht * 128
                    nc.tensor.matmul(pp[:], w1t[:, c0:c0 + 128],
                                     z[dt][:, ch * CW:(ch + 1) * CW],
                                     start=(dt == 0), stop=(dt == DT - 1))
                htile = hfp.tile([128, CW], BF16, name=f"hf{ht}")
                nc.scalar.activation(htile[:], pp[:], AF.Relu,
                                     bias=bcol(COL_B1 + l * 8 + ht))
                hts.append(htile)
            return partial_ar(hts, w2t, HT, ch)

        ckv = [None] * L
        ar_prev = None
        for l in range(L):
            aw = wap.tile([128, 3 * EL * DT], F32R, name="aw")
            if l == 0:
                # split the first layer's 3MB qkv-weight load per dt-block
                # so the first projection matmul starts after 0.4MB
                for dt in range(DT):
                    c0 = dt * 3 * EL
                    nc.sync.dma_start(aw[:, c0:c0 + 3 * EL],
                                      attw_s[l][:, c0:c0 + 3 * EL])
            else:
                nc.sync.dma_start(aw[:], attw_s[l])
            wo_s_t = wop.tile([128, ET * D], BF16, name="wot")
            nc.sync.dma_start(wo_s_t[:], wo_s[l])
            if l == 0:
                # encoder activations resident in bf16 (cross-attn k/v
                # source); loaded after layer-0 weights so the first
                # projections aren't starved behind this 1MB transfer
                for dt in range(DT):
                    et_ = encp.tile([128, TB], BF16, name=f"enc{dt}")
                    nc.sync.dma_start(et_[:], enct[dt * 128:(dt + 1) * 128, :])
                    encs.append(et_)

            # self attention, chunked
            kt = [ktp.tile([128, TB], BF16, name=f"kt{et}")
                  for et in range(ET)]
            sva = [None] * KB
            ar_s = []
            for ch in range(CH):
                if ar_prev is not None:
                    boundary(ar_prev[ch], 3 * l - 1, ch)
                qt, vt = [], []
                for et in range(ET):
                    ppq = zproj(lambda dt: aw[:, dt * 768 + et * 128:
                                              dt * 768 + et * 128 + 128], ch)
                    q = qkvp.tile([128, CW], BF16, name=f"qt{et}")
                    nc.scalar.activation(q[:], ppq[:], AF.Identity,
                                         bias=bcol(COL_QKV + l * 12 + et))
                    qt.append(q)
                    ppk = zproj(lambda dt: aw[:, dt * 768 + 256 + et * 128:
                                              dt * 768 + 256 + et * 128 + 128],
                                ch)
                    nc.scalar.activation(kt[et][:, ch * CW:(ch + 1) * CW],
                                         ppk[:], AF.Identity,
                                         bias=bcol(COL_QKV + l * 12 + 2 + et))
                    ppv = zproj(lambda dt: aw[:, dt * 768 + 512 + et * 128:
                                              dt * 768 + 512 + et * 128 + 128],
                                ch)
                    v = qkvp.tile([128, CW], BF16, name=f"vt{et}")
                    nc.scalar.activation(v[:], ppv[:], AF.Identity,
                                         bias=bcol(COL_QKV + l * 12 + 4 + et))
                    vt.append(v)
                for j in range(2):
                    kb = ch * 2 + j
                    va = svap.tile([128, 384], BF16, name=f"sva{kb}")
                    for et in range(ET):
                        va_fill(va, vt[et], j, et)
                    sva[kb] = va
                hd = attn_core(qt, kt, sva, ch, True, "s")
                ar_s.append(partial_ar(hd, wo_s_t, ET, ch))

            if l == 0:
                # precompute cross k/v for ALL layers while the first
                # collective waits out the inter-core startup barrier
                for ll in range(L):
                    ckv[ll] = cross_kv(ll)

            aqc = waqc.tile([128, EL * DT], F32R, name="aqc")
            nc.sync.dma_start(aqc[:], attq_c[l])
            wo_c_t = wop.tile([128, ET * D], BF16, name="woc")
            nc.sync.dma_start(wo_c_t[:], wo_c[l])

            ar_c = []
            for ch in range(CH):
                boundary(ar_s[ch], 3 * l, ch)
                qtc = []
                for et in range(ET):
                    ppq = zproj(lambda dt: aqc[:, dt * 256 + et * 128:
                                               dt * 256 + et * 128 + 128], ch)
                    q = qkvp.tile([128, CW], BF16, name=f"qc{et}")
                    nc.scalar.activation(q[:], ppq[:], AF.Identity,
                                         bias=bcol(COL_QKV + l * 12 + 6 + et))
                    qtc.append(q)
                hd = attn_core(qtc, ckv[l][0], ckv[l][1], ch, False, "c")
                ar_c.append(partial_ar(hd, wo_c_t, ET, ch))

            w1t = w1pool.tile([128, FFL * DT], F32R, name="w1t")
            nc.sync.dma_start(w1t[:], w1p[l])
            w2t = w2pool.tile([128, D * HT], BF16, name="w2t")
            nc.sync.dma_start(w2t[:], w2p[l])
            ar_f = []
            for ch in range(CH):
                boundary(ar_c[ch], 3 * l + 1, ch)
                ar_f.append(ffn(l, w1t, w2t, ch))
            ar_prev = ar_f

        for ch in range(CH):
            boundary(ar_prev[ch], 3 * L - 1, ch, last=True)

        # vocab projection.  First slices run on chunk A only so the tensor
        # engine starts right after the chunk-A final boundary (chunk B's
        # AllReduce still in flight); the rest run with the full 512-token
        # moving dim to amortize stationary loads.
        NSPLIT = 12
        for vs in range(NSPLIT):
            wt = wvp.tile([128, D], BF16, name="wv", bufs=3)
            nc.sync.dma_start(wt[:], woutp[:, vs * D:(vs + 1) * D])
            pp = ps.tile([128, CW], F32, name="pv", tag="mm")
            for dt in range(DT):
                nc.tensor.matmul(pp[:], wt[:, dt * 128:(dt + 1) * 128],
                                 zb[dt][:, 0:CW], start=(dt == 0),
                                 stop=(dt == DT - 1))
            osb = osp.tile([128, CW], BF16, name="osba", bufs=4)
            if vs % 2 == 0:
                nc.scalar.activation(osb[:], pp[:], AF.Identity,
                                     bias=bcol(COL_BOUT + vs))
            else:
                nc.vector.tensor_scalar_add(osb[:], pp[:],
                                            bcol(COL_BOUT + vs))
            nc.sync.dma_start(logt[vs * 128:(vs + 1) * 128, 0:CW], osb[:])
        for vs in range(VS):
            wt = wvp.tile([128, D], BF16, name="wv", bufs=3)
            nc.sync.dma_start(wt[:], woutp[:, vs * D:(vs + 1) * D])
            c0, cn = (CW, CW) if vs < NSPLIT else (0, TB)
            pp = ps.tile([128, cn], F32, name="pv", tag="mm")
            for dt in range(DT):
                nc.tensor.matmul(pp[:], wt[:, dt * 128:(dt + 1) * 128],
                                 zb[dt][:, c0:c0 + cn], start=(dt == 0),
                                 stop=(dt == DT - 1))
            osb = osp.tile([128, cn], BF16, name="osb")
            if vs % 2 == 0:
                nc.scalar.activation(osb[:], pp[:], AF.Identity,
                                     bias=bcol(COL_BOUT + vs))
            else:
                nc.vector.tensor_scalar_add(osb[:], pp[:],
                                            bcol(COL_BOUT + vs))
            nc.sync.dma_start(logt[vs * 128:(vs + 1) * 128, c0:c0 + cn],
                              osb[:])
    nc.compile()
    return nc


def _host_prepare(inputs):
    """Fold BN/biases into weights, shard per (group, rank); per-core maps."""
    f = lambda a: np.asarray(a, dtype=np.float64)
    tobf = lambda a: a.astype(ml_dtypes.bfloat16)
    seq = np.asarray(inputs["sequence"])
    emb = np.asarray(inputs["emb"], dtype=np.float32)
    pes = np.asarray(inputs["pes"], dtype=np.float32)
    enc = np.asarray(inputs["encoder_out"], dtype=np.float32)

    x0 = emb[seq] + pes[None, :, :]                   # [B, S, D] fp32
    xtg = [np.ascontiguousarray(x0[g].T.astype(np.float32))
           for g in range(B)]                         # [D, TB]
    enctg = [np.ascontiguousarray(tobf(enc[g].T.astype(np.float32)))
             for g in range(B)]

    mask = (np.arange(128)[:, None] < np.arange(128)[None, :])
    maskd = np.ascontiguousarray(tobf(mask.astype(np.float32)))

    attw_s = np.zeros((TPC, L, 128, 3 * EL * DT), np.float32)
    attq_c = np.zeros((TPC, L, 128, EL * DT), np.float32)
    attkv_c = np.zeros((TPC, L, 128, 2 * EL * DT), ml_dtypes.bfloat16)
    wo_s_p = np.zeros((TPC, L, 128, ET * D), ml_dtypes.bfloat16)
    wo_c_p = np.zeros((TPC, L, 128, ET * D), ml_dtypes.bfloat16)
    w1pp = np.zeros((TPC, L, 128, FFL * DT), np.float32)
    w2pp = np.zeros((TPC, L, 128, D * HT), ml_dtypes.bfloat16)
    woutpp = np.zeros((TPC, 128, VS * D), ml_dtypes.bfloat16)
    biaspp = np.zeros((TPC, 128, NBCOL), np.float32)

    def pack_kxm(w, ncols):
        kt = w.shape[0] // 128
        return w.reshape(kt, 128, ncols).transpose(1, 0, 2).reshape(
            128, kt * ncols)

    sig = np.ones(D)
    gam = np.zeros(D)
    for l in range(L):
        for which, (wq, bq, wk, bk, wv, bv, wo, bo, g, be, m, v) in enumerate([
            (inputs["wq_s"][l], inputs["bq_s"][l], inputs["wk_s"][l],
             inputs["bk_s"][l], inputs["wv_s"][l], inputs["bv_s"][l],
             inputs["wo_s"][l], inputs["bo_s"][l], inputs["g1"][l],
             inputs["be1"][l], inputs["m1"][l], inputs["v1"][l]),
            (inputs["wq_c"][l], inputs["bq_c"][l], inputs["wk_c"][l],
             inputs["bk_c"][l], inputs["wv_c"][l], inputs["bv_c"][l],
             inputs["wo_c"][l], inputs["bo_c"][l], inputs["g2"][l],
             inputs["be2"][l], inputs["m2"][l], inputs["v2"][l]),
        ]):
            wq, wk, wv = f(wq), f(wk), f(wv)          # [H, D, DH]
            bq, bk, bv = f(bq), f(bk), f(bv)          # [H, DH]
            wo, bo = f(wo), f(bo)
            for r in range(TPC):
                h0 = r * HL
                wql = wq[h0:h0 + HL].transpose(1, 0, 2).reshape(D, EL)
                wkl = wk[h0:h0 + HL].transpose(1, 0, 2).reshape(D, EL)
                wvl = wv[h0:h0 + HL].transpose(1, 0, 2).reshape(D, EL)
                bql = bq[h0:h0 + HL].reshape(EL)
                bkl = bk[h0:h0 + HL].reshape(EL)
                bvl = bv[h0:h0 + HL].reshape(EL)
                wq_eff = (sig[:, None] * wql) / 8.0
                bq_eff = (gam @ wql + bql) / 8.0
                if which == 0:
                    wk_eff = sig[:, None] * wkl
                    bk_eff = gam @ wkl + bkl
                    wv_eff = sig[:, None] * wvl
                    bv_eff = gam @ wvl + bvl
                    wcat = np.concatenate([wq_eff, wk_eff, wv_eff], axis=1)
                    attw_s[r, l] = pack_kxm(wcat, 3 * EL).astype(np.float32)
                else:
                    # cross k/v read the raw encoder output
                    attq_c[r, l] = pack_kxm(wq_eff, EL).astype(np.float32)
                    kvcat = np.concatenate([wkl, wvl], axis=1)
                    attkv_c[r, l] = tobf(
                        pack_kxm(kvcat, 2 * EL).astype(np.float32))
                    bk_eff, bv_eff = bkl, bvl
                wo_loc = wo[r * EL:(r + 1) * EL, :]
                (wo_s_p if which == 0 else wo_c_p)[r, l] = tobf(
                    pack_kxm(wo_loc, D).astype(np.float32))
                base = 0 if which == 0 else 3
                for et in range(ET):
                    sl = slice(et * 128, (et + 1) * 128)
                    cb = COL_QKV + l * 12
                    biaspp[r, :, cb + (base + 0) * 2 + et] = \
                        bq_eff[sl].astype(np.float32)
                    biaspp[r, :, cb + (base + 1) * 2 + et] = \
                        bk_eff[sl].astype(np.float32)
                    biaspp[r, :, cb + (base + 2) * 2 + et] = \
                        bv_eff[sl].astype(np.float32)
            bnd = 3 * l + which
            for r in range(TPC):
                for dt in range(DT):
                    biaspp[r, :, COL_SIG + bnd * 8 + dt] = \
                        sig[dt * 128:(dt + 1) * 128].astype(np.float32)
            beta = gam + bo
            s = f(g) / np.sqrt(f(v) + EPS)
            cshift = f(be) - f(m) * s
            sig = s
            gam = s * beta + cshift

        # FFN
        w1, b1 = f(inputs["w1"][l]), f(inputs["b1"][l])
        w2, b2 = f(inputs["w2"][l]), f(inputs["b2"][l])
        for r in range(TPC):
            cols = slice(r * FFL, (r + 1) * FFL)
            w1_eff = sig[:, None] * w1[:, cols]
            b1_eff = gam @ w1[:, cols] + b1[cols]
            w1pp[r, l] = pack_kxm(w1_eff, FFL).astype(np.float32)
            w2pp[r, l] = tobf(pack_kxm(w2[cols, :], D).astype(np.float32))
            for ht in range(HT):
                biaspp[r, :, COL_B1 + l * 8 + ht] = \
                    b1_eff[ht * 128:(ht + 1) * 128].astype(np.float32)
        bnd = 3 * l + 2
        for r in range(TPC):
            for dt in range(DT):
                biaspp[r, :, COL_SIG + bnd * 8 + dt] = \
                    sig[dt * 128:(dt + 1) * 128].astype(np.float32)
        beta = gam + b2
        s = f(inputs["g3"][l]) / np.sqrt(f(inputs["v3"][l]) + EPS)
        cshift = f(inputs["be3"][l]) - f(inputs["m3"][l]) * s
        sig = s
        gam = s * beta + cshift

    wout, bout = f(inputs["w_out"]), f(inputs["b_out"])
    for r in range(TPC):
        wsl = np.zeros((D, VPAD))
        bsl = np.zeros(VPAD)
        cols = slice(r * VL, (r + 1) * VL)
        wsl[:, :VL] = wout[:, cols]
        bsl[:VL] = bout[cols]
        wout_eff = sig[:, None] * wsl
        bout_eff = gam @ wsl + bsl
        woutpp[r] = tobf(wout_eff.reshape(DT, 128, VS, 128).transpose(
            1, 2, 0, 3).reshape(128, VS * D).astype(np.float32))
        for vs in range(VS):
            biaspp[r, :, COL_BOUT + vs] = \
                bout_eff[vs * 128:(vs + 1) * 128].astype(np.float32)

    biaspp[:, :, COL_EPS] = 1e-30
    in_maps = []
    for c in range(NC):
        g, r = c // TPC, c % TPC
        in_maps.append({
            "xt": xtg[g], "enct": enctg[g],
            "attw_s": attw_s[r], "attq_c": attq_c[r], "attkv_c": attkv_c[r],
            "wo_s": wo_s_p[r], "wo_c": wo_c_p[r],
            "w1p": w1pp[r], "w2p": w2pp[r], "woutp": woutpp[r],
            "biasp": biaspp[r], "maskd": maskd,
            "identd": tobf(np.eye(128, dtype=np.float32)),
            "onesd": np.ones((128, 64), dtype=ml_dtypes.bfloat16),
        })
    return in_maps


EPS = 1e-3

_NC_CACHE = {}


def _get_program():
    if "nc" not in _NC_CACHE:
        _NC_CACHE["nc"] = _build_program()
    return _NC_CACHE["nc"]


def run(inputs, trace=False):
    nc = _get_program()
    in_maps = _host_prepare(inputs)
    res = bass_utils.run_bass_kernel_spmd(nc, in_maps, list(range(NC)),
                                          trace=trace)
    outs = []
    for g in range(B):
        parts = [np.asarray(res.results[g * TPC + r]["logt"][:VL, :],
                            dtype=np.float32) for r in range(TPC)]
        outs.append(np.concatenate(parts, axis=0).T)   # [TB, V]
    out = np.stack(outs, axis=0).astype(np.float32)    # [B, S, V]
    return out, res


def kernel(**inputs):
    out, _ = run(inputs)
    return out
